# revision 2
# baseline (speedup 1.0000x reference)
"""CoAttention kernel for Trainium2, data-parallel over batch across 8 cores.

Math (per batch element, C:[1024,512], Q:[128,512], W0=[w_c|w_q|w_cq]):
  S[c,q]  = C@w_c[c] + Q@w_q[q] + (C*w_cq)@Q^T [c,q]
  S_q     = softmax_q(S + qbias[q]);  S_c = softmax_c(S^T + cbias[c])
  A       = S_q @ Q;  T = S_c @ C;  Bm = S_q @ T
  out     = [C, A, C*A, C*Bm]

Kernel decomposition (per core, 4 batch elements):
  bilinearT[q,c] = sum_d (Q*w_cq)[q,d] * C[c,d]   (PE, contract d; host
                   supplies C^T and Q^T so d lands on partitions)
  E [q,c]  = exp(bilinearT + (qw+qbias)[q])       (rank-1 cw term cancels in
             softmax_q; qw per-partition bias on ACT)
  denq[c]  = sum_q E  (ones-column tiny matmuls)
  A[c,d]   = (lhsT=E_j, rhs=Q) / denq             (normalization on drain)
  F0[q,c]  = exp(bilinearT - rowmax[q])           (qw term cancels in
             softmax_c; g=exp(cw+cbias) applied on the F0^T drain)
  T[q,d]   = (lhsT=(F0^T*g)_j, rhs=C_j) / denc;  denc via ones-column MMs
  Bm[c,d]  = (lhsT=E_j, rhs=T) / denq
"""
import sys
for p in ('/opt/trn_rl_repo', '/root/.axon_site/_ro/trn_rl_repo'):
    if p not in sys.path:
        sys.path.append(p)

import numpy as np
import concourse.bacc as bacc
import concourse.tile as tile
from concourse import mybir
from concourse.bass_utils import run_bass_kernel_spmd

F32 = mybir.dt.float32
EXP = mybir.ActivationFunctionType.Exp
COPY = mybir.ActivationFunctionType.Copy
AX = mybir.AxisListType.X

B, C_L, Q_L, D = 32, 1024, 128, 512
NCORES = 8
BPC = B // NCORES          # batch elements per core
NJ = C_L // 128            # 8 c-tiles
NK = D // 128              # 4 d-chunks
NEG = -1e7

# consts_sb layout (free-axis offsets)
OFF_ID = 0                 # identity [128, 128]
OFF_WCQ = 128              # w_cq as [128, 4] (chunk k in col k)
OFF_WC = 132               # w_c broadcast [128, 512]
OFF_WQ = 644               # w_q broadcast [128, 512]
NCONST = 1156

_CACHE = {}


def _build():
    nc = bacc.Bacc('TRN2', debug=False, num_devices=NCORES)
    Cd = nc.declare_dram_parameter("Cd", [BPC, C_L, D], F32, isOutput=False)
    CTd = nc.declare_dram_parameter("CTd", [BPC, D, C_L], F32, isOutput=False)
    Qd = nc.declare_dram_parameter("Qd", [BPC, Q_L, D], F32, isOutput=False)
    QTd = nc.declare_dram_parameter("QTd", [BPC, D, Q_L], F32, isOutput=False)
    CON = nc.declare_dram_parameter("CON", [128, NCONST], F32, isOutput=False)
    BIA = nc.declare_dram_parameter("BIA", [BPC, 9 * 128], F32, isOutput=False)
    OUT = nc.declare_dram_parameter("OUT", [BPC, C_L, 4 * D], F32, isOutput=True)

    with tile.TileContext(nc) as tc:
        with tc.tile_pool(name="con", bufs=1) as conp, \
             tc.tile_pool(name="big", bufs=2) as big, \
             tc.tile_pool(name="mid", bufs=2) as mid, \
             tc.tile_pool(name="sml", bufs=2) as sml, \
             tc.tile_pool(name="outp", bufs=3) as outp, \
             tc.tile_pool(name="ps_s", bufs=1, space="PSUM") as ps_s, \
             tc.tile_pool(name="ps_tr", bufs=2, space="PSUM") as ps_tr, \
             tc.tile_pool(name="ps_t", bufs=1, space="PSUM") as ps_t, \
             tc.tile_pool(name="ps_den", bufs=1, space="PSUM") as ps_den, \
             tc.tile_pool(name="ps_ab", bufs=2, space="PSUM") as ps_ab:

            consts = conp.tile([128, NCONST], F32)
            nc.sync.dma_start(out=consts, in_=CON[:, :])
            ident = consts[:, OFF_ID:OFF_ID + 128]
            ones_col = conp.tile([128, 1], F32)
            nc.vector.memset(ones_col, 1.0)

            for b in range(BPC):
                # ---- input DMAs (one per tensor: single HWDGE lane each)
                c_sb = big.tile([128, NJ, 512], F32, tag="c")
                nc.sync.dma_start(
                    out=c_sb, in_=Cd[b].rearrange("(j p) d -> p j d", p=128))
                ct_sb = big.tile([128, NK, 1024], F32, tag="ct")
                nc.sync.dma_start(
                    out=ct_sb, in_=CTd[b].rearrange("(k p) c -> p k c", p=128))
                q_sb = mid.tile([128, 512], F32, tag="q")
                nc.sync.dma_start(out=q_sb, in_=Qd[b][:, :])
                qt_sb = mid.tile([128, NK, 128], F32, tag="qt")
                nc.sync.dma_start(
                    out=qt_sb, in_=QTd[b].rearrange("(k p) q -> p k q", p=128))
                bias_sb = sml.tile([128, 9], F32, tag="bias")
                nc.sync.dma_start(
                    out=bias_sb, in_=BIA[b].rearrange("(j p) -> p j", p=128))
                # bias_sb col 0 = qbias, cols 1..8 = cbias chunks

                # ---- prep: QW^T = QT * w_cq (per-partition scalar per chunk)
                qwt_sb = mid.tile([128, NK, 128], F32, tag="qwt")
                for k in range(NK):
                    nc.vector.tensor_scalar_mul(
                        qwt_sb[:, k, :], qt_sb[:, k, :],
                        consts[:, OFF_WCQ + k:OFF_WCQ + k + 1])

                # qw' = rowsum(Q * w_q_bcast) + qbias
                qmul = mid.tile([128, 512], F32, tag="qmul")
                nc.gpsimd.tensor_mul(qmul, q_sb, consts[:, OFF_WQ:OFF_WQ + 512])
                qwp = sml.tile([128, 1], F32, tag="qwp")
                nc.vector.reduce_sum(out=qwp, in_=qmul, axis=AX)
                nc.vector.tensor_scalar_add(qwp, qwp, bias_sb[:, 0:1])

                # cw' = rowsum(C_j * w_c_bcast) + cbias;  g = exp(cw')
                cw_sb = sml.tile([128, 8], F32, tag="cw")
                for j in range(NJ):
                    cmul = mid.tile([128, 512], F32, tag="cmul")
                    nc.gpsimd.tensor_mul(
                        cmul, c_sb[:, j, :], consts[:, OFF_WC:OFF_WC + 512])
                    nc.vector.reduce_sum(out=cw_sb[:, j:j + 1], in_=cmul, axis=AX)
                nc.vector.tensor_add(cw_sb, cw_sb, bias_sb[:, 1:9])
                g_sb = sml.tile([128, 8], F32, tag="g")
                nc.scalar.activation(out=g_sb, in_=cw_sb, func=EXP)

                # ---- bilinearT [q=128, c=1024] on PE, accumulate over d
                s_ps = ps_s.tile([128, 1024], F32)
                for n in range(2):
                    for k in range(NK):
                        nc.tensor.matmul(
                            s_ps[:, n * 512:(n + 1) * 512],
                            qwt_sb[:, k, :],
                            ct_sb[:, k, n * 512:(n + 1) * 512],
                            start=(k == 0), stop=(k == NK - 1))

                # ---- E = exp(S^T + qw'), F0 = exp(S^T - rowmax)
                e_sb = big.tile([128, 1024], F32, tag="e")
                nc.scalar.activation(out=e_sb, in_=s_ps, func=EXP, bias=qwp)
                negmax = sml.tile([128, 1], F32, tag="negmax")
                nc.vector.reduce_max(out=negmax, in_=s_ps, axis=AX, negate=True)
                f0_sb = big.tile([128, 1024], F32, tag="f0")
                nc.scalar.activation(out=f0_sb, in_=s_ps, func=EXP, bias=negmax)

                # ---- F0^T chunks, scaled by g -> FGT [c-part, q-free] per j
                fgt_sb = big.tile([128, NJ, 128], F32, tag="fgt")
                for j in range(NJ):
                    tr_ps = ps_tr.tile([128, 128], F32, tag="tr")
                    nc.tensor.transpose(
                        tr_ps, f0_sb[:, j * 128:(j + 1) * 128], ident)
                    nc.vector.tensor_scalar_mul(
                        fgt_sb[:, j, :], tr_ps, g_sb[:, j:j + 1])

                # ---- T = sum_j FGT_j^T @ C_j ; denc via ones column
                t_ps = ps_t.tile([128, 512], F32)
                den_ps = ps_den.tile([128, 16], F32)
                for j in range(NJ):
                    nc.tensor.matmul(
                        t_ps, fgt_sb[:, j, :], c_sb[:, j, :],
                        start=(j == 0), stop=(j == NJ - 1))
                    nc.tensor.matmul(
                        den_ps[:, 0:1], fgt_sb[:, j, :], ones_col,
                        start=(j == 0), stop=(j == NJ - 1))
                rdc = sml.tile([128, 1], F32, tag="rdc")
                nc.vector.reciprocal(rdc, den_ps[:, 0:1])
                t_sb = mid.tile([128, 512], F32, tag="t")
                nc.scalar.activation(out=t_sb, in_=t_ps, func=COPY, scale=rdc)

                # ---- A + B phase per j (B needs t_sb, ready above)
                rdq = sml.tile([128, 8], F32, tag="rdq")
                for j in range(NJ):
                    e_j = e_sb[:, j * 128:(j + 1) * 128]
                    a_ps = ps_ab.tile([128, 512], F32, tag="ab")
                    nc.tensor.matmul(a_ps, e_j, q_sb, start=True, stop=True)
                    nc.tensor.matmul(
                        den_ps[:, 8 + j:9 + j], e_j, ones_col,
                        start=True, stop=True)
                    b_ps = ps_ab.tile([128, 512], F32, tag="ab")
                    nc.tensor.matmul(b_ps, e_j, t_sb, start=True, stop=True)
                    nc.vector.reciprocal(rdq[:, j:j + 1], den_ps[:, 8 + j:9 + j])
                    o_sb = outp.tile([128, 1536], F32, tag="o")
                    nc.scalar.activation(
                        out=o_sb[:, 0:512], in_=a_ps, func=COPY,
                        scale=rdq[:, j:j + 1])
                    nc.vector.tensor_mul(
                        o_sb[:, 512:1024], c_sb[:, j, :], o_sb[:, 0:512])
                    bn = mid.tile([128, 512], F32, tag="bn")
                    nc.scalar.activation(
                        out=bn, in_=b_ps, func=COPY, scale=rdq[:, j:j + 1])
                    nc.vector.tensor_mul(
                        o_sb[:, 1024:1536], c_sb[:, j, :], bn)
                    # ---- stores
                    orow = OUT[b].rearrange(
                        "(jj p) f -> p jj f", p=128)[:, j, :]
                    nc.sync.dma_start(out=orow[:, 512:2048], in_=o_sb)
                    nc.sync.dma_start(out=orow[:, 0:512], in_=c_sb[:, j, :])

    nc.compile()
    return nc


def kernel(context_embed, question_embed, context_mask, question_mask, W0):
    C = np.ascontiguousarray(context_embed, dtype=np.float32)
    Q = np.ascontiguousarray(question_embed, dtype=np.float32)
    W0 = np.asarray(W0, dtype=np.float32)
    w_c, w_q, w_cq = W0[:D], W0[D:2 * D], W0[2 * D:]

    if 'nc' not in _CACHE:
        _CACHE['nc'] = _build()
    nc = _CACHE['nc']

    CT = np.ascontiguousarray(C.transpose(0, 2, 1))           # [B, D, C_L]
    QT = np.ascontiguousarray(Q.transpose(0, 2, 1))           # [B, D, Q_L]

    con = np.zeros((128, NCONST), dtype=np.float32)
    con[:, OFF_ID:OFF_ID + 128] = np.eye(128, dtype=np.float32)
    con[:, OFF_WCQ:OFF_WCQ + NK] = w_cq.reshape(NK, 128).T
    con[:, OFF_WC:OFF_WC + 512] = np.broadcast_to(w_c, (128, 512))
    con[:, OFF_WQ:OFF_WQ + 512] = np.broadcast_to(w_q, (128, 512))

    qbias = np.where(question_mask, 0.0, NEG).astype(np.float32)  # [B, 128]
    cbias = np.where(context_mask, 0.0, NEG).astype(np.float32)   # [B, 1024]
    bia = np.concatenate([qbias, cbias], axis=1)                  # [B, 1152]

    core_ids = list(range(NCORES))
    in_maps = []
    for i in core_ids:
        s = slice(i * BPC, (i + 1) * BPC)
        in_maps.append({
            "Cd": C[s], "CTd": CT[s], "Qd": Q[s], "QTd": QT[s],
            "CON": con, "BIA": np.ascontiguousarray(bia[s]),
        })
    res = run_bass_kernel_spmd(nc, in_maps, core_ids)
    out = np.concatenate([res.results[i]["OUT"] for i in core_ids], axis=0)
    return out


# revision 6
# speedup vs baseline: 1.0941x; 1.0941x over previous
"""CoAttention kernel for Trainium2, data-parallel over batch across 8 cores.

Math (per batch element, C:[1024,512], Q:[128,512], W0=[w_c|w_q|w_cq]):
  S[c,q]  = C@w_c[c] + Q@w_q[q] + (C*w_cq)@Q^T [c,q]
  S_q     = softmax_q(S + qbias[q]);  S_c = softmax_c(S^T + cbias[c])
  A       = S_q @ Q;  T = S_c @ C;  Bm = S_q @ T
  out     = [C, A, C*A, C*Bm]

Kernel decomposition (per core, 4 batch elements):
  bilinearT[q,c] = sum_d (Q*w_cq)[q,d] * C[c,d]   (PE fp32, contract d; host
                   supplies C^T and Q^T so d lands on partitions)
  E [q,c]  = exp(bilinearT + (qw+qbias)[q])       (rank-1 cw term cancels in
             softmax_q; qw per-partition bias on ACT). Stored f32r.
  denq[c]  = sum_q E  (ones-column tiny matmuls)
  A[c,d]   = (lhsT=E_j, rhs=Q) / denq             (f32r MM; norm on drain)
  F0[q,c]  = exp(bilinearT - rowmax[q])           (qw term cancels in
             softmax_c; g=exp(cw+cbias) applied on the F0^T drain)
  T[q,d]   = (lhsT=(F0^T*g)_j, rhs=C_j) / denc    (fp32; denc via ones MMs)
  Bm[c,d]  = (lhsT=E_j, rhs=T) / denq             (f32r MM)

A/B matmuls use float32r (PE runs 4x faster): they only average positive
softmax weights against Q/T, so the ~1e-4 rounding is benign. The S matmul
(pre-exp) stays fp32.
"""
import sys
for p in ('/opt/trn_rl_repo', '/root/.axon_site/_ro/trn_rl_repo'):
    if p not in sys.path:
        sys.path.append(p)

import numpy as np
import concourse.bacc as bacc
import concourse.tile as tile
from concourse import mybir
from concourse.bass_utils import run_bass_kernel_spmd

F32 = mybir.dt.float32
F32R = mybir.dt.float32r
EXP = mybir.ActivationFunctionType.Exp
COPY = mybir.ActivationFunctionType.Copy
AX = mybir.AxisListType.X

B, C_L, Q_L, D = 32, 1024, 128, 512
NCORES = 8
BPC = B // NCORES          # batch elements per core
NJ = C_L // 128            # 8 c-tiles
NK = D // 128              # 4 d-chunks
NEG = -1e7

# consts_sb layout (free-axis offsets)
OFF_ID = 0                 # identity [128, 128]
OFF_WCQ = 128              # w_cq as [128, 4] (chunk k in col k)
OFF_WC = 132               # w_c broadcast [128, 512]
OFF_WQ = 644               # w_q broadcast [128, 512]
NCONST = 1156

_CACHE = {}


def _build():
    nc = bacc.Bacc('TRN2', debug=False, num_devices=NCORES,
                   num_swdge_queues=4)
    Cd = nc.declare_dram_parameter("Cd", [BPC, C_L, D], F32, isOutput=False)
    CTd = nc.declare_dram_parameter("CTd", [BPC, D, C_L], F32, isOutput=False)
    Qd = nc.declare_dram_parameter("Qd", [BPC, Q_L, D], F32, isOutput=False)
    QTd = nc.declare_dram_parameter("QTd", [BPC, D, Q_L], F32, isOutput=False)
    CON = nc.declare_dram_parameter("CON", [128, NCONST], F32, isOutput=False)
    BIA = nc.declare_dram_parameter("BIA", [BPC, 9 * 128], F32, isOutput=False)
    OUT = nc.declare_dram_parameter("OUT", [BPC, C_L, 4 * D], F32, isOutput=True)

    with tile.TileContext(nc) as tc:
        with tc.tile_pool(name="con", bufs=1) as conp, \
             tc.tile_pool(name="big", bufs=2) as big, \
             tc.tile_pool(name="mid", bufs=2) as mid, \
             tc.tile_pool(name="sml", bufs=2) as sml, \
             tc.tile_pool(name="outp", bufs=3) as outp, \
             tc.tile_pool(name="ps_s", bufs=1, space="PSUM") as ps_s, \
             tc.tile_pool(name="ps_tr", bufs=2, space="PSUM") as ps_tr, \
             tc.tile_pool(name="ps_t", bufs=1, space="PSUM") as ps_t, \
             tc.tile_pool(name="ps_den", bufs=1, space="PSUM") as ps_den, \
             tc.tile_pool(name="ps_ab", bufs=2, space="PSUM") as ps_ab:

            consts = conp.tile([128, NCONST], F32)
            nc.sync.dma_start(out=consts, in_=CON[:, :])
            ident = consts[:, OFF_ID:OFF_ID + 128]
            ones_col = conp.tile([128, 1], F32)
            nc.vector.memset(ones_col, 1.0)
            ones_r = conp.tile([128, 2], F32R)
            nc.vector.tensor_copy(ones_r[:, 0:1], ones_col)
            nc.vector.tensor_copy(ones_r[:, 1:2], ones_col)

            for b in range(BPC):
                # ---- input DMAs: tensors needed first load first
                qt_sb = mid.tile([128, NK, 128], F32, tag="qt")
                nc.sync.dma_start(
                    out=qt_sb, in_=QTd[b].rearrange("(k p) q -> p k q", p=128))
                ct_sb = big.tile([128, NK, 1024], F32, tag="ct")
                ct_in = CTd[b].rearrange("(k p) c -> p k c", p=128)
                for n in range(2):
                    nc.sync.dma_start(
                        out=ct_sb[:, :, n * 512:(n + 1) * 512],
                        in_=ct_in[:, :, n * 512:(n + 1) * 512])
                q_sb = mid.tile([128, 512], F32, tag="q")
                nc.sync.dma_start(out=q_sb, in_=Qd[b][:, :])
                bias_sb = sml.tile([128, 9], F32, tag="bias")
                nc.sync.dma_start(
                    out=bias_sb, in_=BIA[b].rearrange("(j p) -> p j", p=128))
                # bias_sb col 0 = qbias, cols 1..8 = cbias chunks
                c_sb = big.tile([128, NJ, 512], F32, tag="c")
                nc.sync.dma_start(
                    out=c_sb, in_=Cd[b].rearrange("(j p) d -> p j d", p=128))

                # ---- prep: QW^T = QT * w_cq (per-partition scalar per chunk)
                qwt_sb = mid.tile([128, NK, 128], F32, tag="qwt")
                for k in range(NK):
                    nc.vector.tensor_scalar_mul(
                        qwt_sb[:, k, :], qt_sb[:, k, :],
                        consts[:, OFF_WCQ + k:OFF_WCQ + k + 1])

                # qw' = rowsum(Q * w_q_bcast) + qbias;  qr = round(Q) for A-MM
                qmul = mid.tile([128, 512], F32, tag="qmul")
                nc.gpsimd.tensor_mul(qmul, q_sb, consts[:, OFF_WQ:OFF_WQ + 512])
                qwp = sml.tile([128, 1], F32, tag="qwp")
                nc.vector.reduce_sum(out=qwp, in_=qmul, axis=AX)
                nc.vector.tensor_scalar_add(qwp, qwp, bias_sb[:, 0:1])
                qr_sb = mid.tile([128, 512], F32R, tag="qr")
                nc.scalar.copy(qr_sb, q_sb)

                # cw' = rowsum(C_j * w_c_bcast) + cbias;  g = exp(cw')
                cw_sb = sml.tile([128, 8], F32, tag="cw")
                for j in range(NJ):
                    cmul = mid.tile([128, 512], F32, tag="cmul")
                    nc.gpsimd.tensor_mul(
                        cmul, c_sb[:, j, :], consts[:, OFF_WC:OFF_WC + 512])
                    nc.vector.reduce_sum(out=cw_sb[:, j:j + 1], in_=cmul, axis=AX)
                nc.vector.tensor_add(cw_sb, cw_sb, bias_sb[:, 1:9])
                g_sb = sml.tile([128, 8], F32, tag="g")
                nc.scalar.activation(out=g_sb, in_=cw_sb, func=EXP)

                # ---- bilinearT [q=128, c=1024] on PE fp32, accumulate over d.
                # E/F0 computed per 512-half so transposes start earlier; the
                # softmax_c shift uses the half-0 rowmax (any row constant
                # works for stability).
                s_ps = ps_s.tile([128, 1024], F32)
                e_sb = big.tile([128, 1024], F32R, tag="e")
                f0_sb = big.tile([128, 1024], F32, tag="f0")
                negmax = sml.tile([128, 1], F32, tag="negmax")
                for n in range(2):
                    h = slice(n * 512, (n + 1) * 512)
                    for k in range(NK):
                        nc.tensor.matmul(
                            s_ps[:, h], qwt_sb[:, k, :], ct_sb[:, k, h],
                            start=(k == 0), stop=(k == NK - 1))
                    if n == 0:
                        nc.vector.reduce_max(
                            out=negmax, in_=s_ps[:, h], axis=AX, negate=True)
                    nc.scalar.activation(
                        out=e_sb[:, h], in_=s_ps[:, h], func=EXP, bias=qwp)
                    nc.scalar.activation(
                        out=f0_sb[:, h], in_=s_ps[:, h], func=EXP, bias=negmax)

                # ---- F0^T chunks, scaled by g -> FGT [c-part, q-free] per j
                fgt_sb = big.tile([128, NJ, 128], F32, tag="fgt")
                for j in range(NJ):
                    tr_ps = ps_tr.tile([128, 128], F32, tag="tr")
                    nc.tensor.transpose(
                        tr_ps, f0_sb[:, j * 128:(j + 1) * 128], ident)
                    nc.vector.tensor_scalar_mul(
                        fgt_sb[:, j, :], tr_ps, g_sb[:, j:j + 1])

                # ---- T = sum_j FGT_j^T @ C_j ; denc via ones column
                t_ps = ps_t.tile([128, 512], F32)
                den_ps = ps_den.tile([128, 24], F32)
                for j in range(NJ):
                    nc.tensor.matmul(
                        t_ps, fgt_sb[:, j, :], c_sb[:, j, :],
                        start=(j == 0), stop=(j == NJ - 1))
                    nc.tensor.matmul(
                        den_ps[:, 0:1], fgt_sb[:, j, :], ones_col,
                        start=(j == 0), stop=(j == NJ - 1))
                rdc = sml.tile([128, 1], F32, tag="rdc")
                nc.vector.reciprocal(rdc, den_ps[:, 0:1])
                t_sb = mid.tile([128, 512], F32R, tag="t")
                nc.scalar.activation(out=t_sb, in_=t_ps, func=COPY, scale=rdc)

                # ---- A + B phase per j (f32r matmuls)
                rdq = sml.tile([128, 8], F32, tag="rdq")
                for j in range(NJ):
                    e_j = e_sb[:, j * 128:(j + 1) * 128]
                    a_ps = ps_ab.tile([128, 512], F32, tag="ab")
                    nc.tensor.matmul(a_ps, e_j, qr_sb, start=True, stop=True)
                    nc.tensor.matmul(
                        den_ps[:, 8 + 2 * j:10 + 2 * j], e_j, ones_r,
                        start=True, stop=True)
                    b_ps = ps_ab.tile([128, 512], F32, tag="ab")
                    nc.tensor.matmul(b_ps, e_j, t_sb, start=True, stop=True)
                    nc.vector.reciprocal(rdq[:, j:j + 1], den_ps[:, 8 + 2 * j:9 + 2 * j])
                    o_sb = outp.tile([128, 1536], F32, tag="o")
                    nc.scalar.activation(
                        out=o_sb[:, 0:512], in_=a_ps, func=COPY,
                        scale=rdq[:, j:j + 1])
                    nc.vector.tensor_mul(
                        o_sb[:, 512:1024], c_sb[:, j, :], o_sb[:, 0:512])
                    bn = mid.tile([128, 512], F32, tag="bn")
                    nc.scalar.activation(
                        out=bn, in_=b_ps, func=COPY, scale=rdq[:, j:j + 1])
                    nc.vector.tensor_mul(
                        o_sb[:, 1024:1536], c_sb[:, j, :], bn)
                    # store A|CA|CB via SWDGE (keeps SP free for input loads)
                    orow = OUT[b].rearrange(
                        "(jj p) f -> p jj f", p=128)[:, j, :]
                    nc.gpsimd.dma_start(out=orow[:, 512:2048], in_=o_sb)
                # copy-through C in one batched store
                nc.gpsimd.dma_start(
                    out=OUT[b].rearrange("(j p) f -> p j f", p=128)[:, :, 0:512],
                    in_=c_sb)

    nc.compile()
    return nc


def kernel(context_embed, question_embed, context_mask, question_mask, W0):
    C = np.ascontiguousarray(context_embed, dtype=np.float32)
    Q = np.ascontiguousarray(question_embed, dtype=np.float32)
    W0 = np.asarray(W0, dtype=np.float32)
    w_c, w_q, w_cq = W0[:D], W0[D:2 * D], W0[2 * D:]

    if 'nc' not in _CACHE:
        _CACHE['nc'] = _build()
    nc = _CACHE['nc']

    CT = np.ascontiguousarray(C.transpose(0, 2, 1))           # [B, D, C_L]
    QT = np.ascontiguousarray(Q.transpose(0, 2, 1))           # [B, D, Q_L]

    con = np.zeros((128, NCONST), dtype=np.float32)
    con[:, OFF_ID:OFF_ID + 128] = np.eye(128, dtype=np.float32)
    con[:, OFF_WCQ:OFF_WCQ + NK] = w_cq.reshape(NK, 128).T
    con[:, OFF_WC:OFF_WC + 512] = np.broadcast_to(w_c, (128, 512))
    con[:, OFF_WQ:OFF_WQ + 512] = np.broadcast_to(w_q, (128, 512))

    qbias = np.where(question_mask, 0.0, NEG).astype(np.float32)  # [B, 128]
    cbias = np.where(context_mask, 0.0, NEG).astype(np.float32)   # [B, 1024]
    bia = np.concatenate([qbias, cbias], axis=1)                  # [B, 1152]

    core_ids = list(range(NCORES))
    in_maps = []
    for i in core_ids:
        s = slice(i * BPC, (i + 1) * BPC)
        in_maps.append({
            "Cd": C[s], "CTd": CT[s], "Qd": Q[s], "QTd": QT[s],
            "CON": con, "BIA": np.ascontiguousarray(bia[s]),
        })
    res = run_bass_kernel_spmd(nc, in_maps, core_ids)
    out = np.concatenate([res.results[i]["OUT"] for i in core_ids], axis=0)
    return out
